# revision 49
# baseline (speedup 1.0000x reference)
"""Bidirectional GQA attention (B=2, S=4096, D=768, H=6, HK=2, HD=128) on 8 TRN2 cores.

Sharding: core c handles batch b = c//4 and query rows [(c%4)*1024, +1024).
Each core computes K/V for its full batch (replicated within the 4-core batch
group), Q for its query slice, full bidirectional attention for its queries
across all 6 heads, and the output projection. No collectives; the host
concatenates the 8 per-core [1024, 768] results.

All matmuls run in float32r (TF32-like, full PE rate at N>=256, ~1e-4 rel err).
Softmax is unmasked and numerically tame here (|scores| < ~3), so exp is taken
directly (no max subtraction); normalization happens after the PV matmul using
an all-ones stationary matmul to produce per-query denominators broadcast
across all 128 partitions.

Layouts (SBUF partition dim first):
  kt_s [128=d, g, tokens]   K^T per kv head  -> QK lhsT
  v_s  [128=tok-subtile, kt, dv]  V natural  -> PV lhsT
  qt_s [128=d, h, queries]  Q^T per head     -> QK rhs; overwritten by O^T
Scores come out key-major ([keys, queries] = S^T), which feeds PV directly
with no transposes anywhere.
"""

import math

import numpy as np

H, HK, HD = 6, 2, 128
B, S, D = 2, 4096, 768
NQ = S // 4          # queries per core
N_CORES = 8
DT = D // 128        # 6 contraction tiles over D
CHUNK = 512          # token chunk for projections
KSUB = S // 128      # 32 key subtiles
SCALE = 1.0 / math.sqrt(HD)

_CACHE: dict = {}


def _build_nc(repeats=1):
    import concourse.bacc as bacc
    import concourse.mybir as mybir
    import concourse.tile as tile

    f32 = mybir.dt.float32
    f32r = mybir.dt.float32r
    Exp = mybir.ActivationFunctionType.Exp

    nc = bacc.Bacc("TRN2", target_bir_lowering=False, debug=False)

    xt = nc.dram_tensor("xt", [D, S], f32r, kind="ExternalInput")
    xtq = nc.dram_tensor("xtq", [D, NQ], f32r, kind="ExternalInput")
    wqt = nc.dram_tensor("wqt", [D, H * HD], f32r, kind="ExternalInput")
    wkt = nc.dram_tensor("wkt", [D, HK * HD], f32r, kind="ExternalInput")
    wvt = nc.dram_tensor("wvt", [D, HK * HD], f32r, kind="ExternalInput")
    wot = nc.dram_tensor("wot", [H * HD, D], f32r, kind="ExternalInput")
    y = nc.dram_tensor("y", [NQ, D], f32, kind="ExternalOutput")

    xt_r = xt.ap().rearrange("(dt p) t -> p dt t", p=128)
    xtq_r = xtq.ap().rearrange("(dt p) t -> p dt t", p=128)
    wqt_r = wqt.ap().rearrange("(dt p) e -> p dt e", p=128)
    wkt_r = wkt.ap().rearrange("(dt p) e -> p dt e", p=128)
    wvt_r = wvt.ap().rearrange("(dt p) e -> p dt e", p=128)
    wot_r = wot.ap().rearrange("(h p) e -> p h e", p=128)

    with tile.TileContext(nc) as tc:
        with (
            tc.tile_pool(name="singles", bufs=1) as singles,
            tc.tile_pool(name="xpool", bufs=3) as xpool,
            tc.tile_pool(name="epool", bufs=4) as epool,
            tc.tile_pool(name="rpool", bufs=3) as rpool,
            tc.tile_pool(name="ypool", bufs=3) as ypool,
            tc.tile_pool(name="ps_sc", bufs=2, space="PSUM") as ps_sc,
            tc.tile_pool(name="ps_pv", bufs=2, space="PSUM") as ps_pv,
            tc.tile_pool(name="ps_dn", bufs=2, space="PSUM") as ps_dn,
        ):
            # --- weights + constants ---
            wq_s = singles.tile([128, DT, H * HD], f32r)
            wk_s = singles.tile([128, DT, HK * HD], f32r)
            wv_s = singles.tile([128, DT, HK * HD], f32r)
            wo_s = singles.tile([128, H, D], f32r)

            ones_f = singles.tile([128, 128], f32)
            nc.vector.memset(ones_f, 1.0)
            ones = singles.tile([128, 128], f32r)
            nc.vector.tensor_copy(out=ones, in_=ones_f)
            # Warm the ACT exp table (~2.7us ACT_TABLE_LOAD) during startup DMA.
            warm = singles.tile([128, 1], f32)
            nc.scalar.activation(out=warm, in_=ones_f[:, 0:1], func=Exp, scale=1.0)

            # --- persistent activations ---
            kt_s = singles.tile([128, HK, S], f32r)         # K^T [d, g, tokens]
            v_s = singles.tile([128, KSUB, HK * HD], f32r)  # V natural [tok, kt, dv]
            qt_s = singles.tile([128, H, NQ], f32r)         # Q^T [d, h, queries]
            # O^T aliases Q^T: head h's Q slice is dead once its QK matmuls
            # have run, and the normalized output has the same shape.
            ot_s = qt_s

            def body():
                # --- phase 1: K/V projections, streamed in 512-token chunks.
                # DMA schedule: critical path is wk -> xc0 (first matmuls),
                # then wv before chunk 0's V projection. wq streams piecewise
                # between x chunks; wo (phase 3 only) after the Q-proj inputs.
                for c in range(S // CHUNK):
                    xc = xpool.tile([128, DT, CHUNK], f32r, tag="xc")
                    if c == 0:
                        for dt in range(DT):
                            nc.sync.dma_start(out=wk_s[:, dt, :], in_=wkt_r[:, dt, :])
                            nc.sync.dma_start(out=xc[:, dt, :], in_=xt_r[:, dt, :CHUNK])
                        # parallel DGE queue: wv streams alongside the x
                        # chunks instead of delaying chunk 1 on the sync queue
                        nc.scalar.dma_start(out=wv_s, in_=wvt_r)
                    elif c <= 3:
                        # early chunks: DMA supply lags PE demand, so split
                        # per D-tile — the first KT matmul of the chunk can
                        # start as soon as its leading tile lands.
                        for dt in range(DT):
                            nc.sync.dma_start(
                                out=xc[:, dt, :],
                                in_=xt_r[:, dt, c * CHUNK:(c + 1) * CHUNK],
                            )
                        if c == 3:
                            nc.sync.dma_start(out=wq_s[:, 0, :], in_=wqt_r[:, 0, :])
                    else:
                        nc.sync.dma_start(
                            out=xc, in_=xt_r[:, :, c * CHUNK:(c + 1) * CHUNK]
                        )
                        dt = c - 3
                        nc.sync.dma_start(out=wq_s[:, dt, :], in_=wqt_r[:, dt, :])
                    for g in range(HK):
                        ps = ps_pv.tile([128, CHUNK], f32, tag="pv")
                        for dt in range(DT):
                            nc.tensor.matmul(
                                ps, wk_s[:, dt, g * 128:(g + 1) * 128], xc[:, dt, :],
                                start=(dt == 0), stop=(dt == DT - 1),
                            )
                        nc.vector.tensor_copy(
                            out=kt_s[:, g, c * CHUNK:(c + 1) * CHUNK], in_=ps
                        )
                    for ts in range(CHUNK // 128):
                        ps = ps_pv.tile([128, HK * HD], f32, tag="pv")
                        for dt in range(DT):
                            nc.tensor.matmul(
                                ps, xc[:, dt, ts * 128:(ts + 1) * 128], wv_s[:, dt, :],
                                start=(dt == 0), stop=(dt == DT - 1),
                            )
                        nc.vector.tensor_copy(out=v_s[:, c * 4 + ts, :], in_=ps)

                # --- phase 1b: Q projection for this core's query slice ---
                # last wq piece (chunks 3..7 carried the first five)
                nc.sync.dma_start(out=wq_s[:, DT - 1, :], in_=wqt_r[:, DT - 1, :])
                for qc in range(NQ // CHUNK):
                    xq = xpool.tile([128, DT, CHUNK], f32r, tag="xc")
                    nc.sync.dma_start(
                        out=xq, in_=xtq_r[:, :, qc * CHUNK:(qc + 1) * CHUNK]
                    )
                    if qc == 1:
                        nc.sync.dma_start(out=wo_s, in_=wot_r)
                    for h in range(H):
                        ps = ps_pv.tile([128, CHUNK], f32, tag="pv")
                        for dt in range(DT):
                            nc.tensor.matmul(
                                ps, wq_s[:, dt, h * 128:(h + 1) * 128], xq[:, dt, :],
                                start=(dt == 0), stop=(dt == DT - 1),
                            )
                        nc.vector.tensor_copy(
                            out=qt_s[:, h, qc * CHUNK:(qc + 1) * CHUNK], in_=ps
                        )

                # --- phase 2: attention per (query chunk, head) ---
                for qc in range(NQ // CHUNK):
                    qsl = slice(qc * CHUNK, (qc + 1) * CHUNK)
                    for h in range(H):
                        g = h // (H // HK)
                        pv = ps_pv.tile([128, CHUNK], f32, tag="pv")
                        dn = ps_dn.tile([128, CHUNK], f32, tag="dn")

                        # software pipeline: QK for pair k2+1 is issued before
                        # PV/ones for k2, so the PE never stalls behind exp.
                        def qk(k2):
                            sc = ps_sc.tile([128, 2 * CHUNK], f32, tag="sc")
                            for j in (0, 1):
                                kt = 2 * k2 + j
                                nc.tensor.matmul(
                                    sc[:, j * CHUNK:(j + 1) * CHUNK],
                                    kt_s[:, g, kt * 128:(kt + 1) * 128],
                                    qt_s[:, h, qsl],
                                    start=True, stop=True,
                                )
                            e = epool.tile([128, 2 * CHUNK], f32r, tag="e")
                            nc.scalar.activation(out=e, in_=sc, func=Exp, scale=SCALE)
                            return e

                        n_iter = KSUB // 2
                        e_cur = qk(0)
                        for k2 in range(n_iter):
                            e_next = qk(k2 + 1) if k2 + 1 < n_iter else None
                            for j in (0, 1):
                                kt = 2 * k2 + j
                                nc.tensor.matmul(
                                    pv, v_s[:, kt, g * 128:(g + 1) * 128],
                                    e_cur[:, j * CHUNK:(j + 1) * CHUNK],
                                    start=(kt == 0), stop=(kt == KSUB - 1),
                                )
                                nc.tensor.matmul(
                                    dn, ones[:],
                                    e_cur[:, j * CHUNK:(j + 1) * CHUNK],
                                    start=(kt == 0), stop=(kt == KSUB - 1),
                                )
                            e_cur = e_next
                        rec = rpool.tile([128, CHUNK], f32, tag="rec")
                        nc.vector.reciprocal(out=rec, in_=dn)
                        nc.vector.tensor_mul(ot_s[:, h, qsl], pv, rec)

                    # --- phase 3: output projection for this query chunk ---
                    for ts in range(CHUNK // 128):
                        ys = ypool.tile([128, D], f32, tag="y")
                        for ec in range(2):
                            esl = slice(ec * 384, (ec + 1) * 384)
                            ps = ps_dn.tile([128, 384], f32, tag="dn")
                            for h in range(H):
                                nc.tensor.matmul(
                                    ps,
                                    ot_s[:, h,
                                         qc * CHUNK + ts * 128:qc * CHUNK + (ts + 1) * 128],
                                    wo_s[:, h, esl],
                                    start=(h == 0), stop=(h == H - 1),
                                )
                            nc.vector.tensor_copy(out=ys[:, esl], in_=ps)
                            r0 = qc * CHUNK + ts * 128
                            nc.sync.dma_start(
                                out=y.ap()[r0:r0 + 128, esl], in_=ys[:, esl]
                            )

            for _ in range(repeats):
                body()

    nc.compile()
    return nc


def _get_nc(repeats=1):
    key = ("nc", repeats)
    if key not in _CACHE:
        _CACHE[key] = _build_nc(repeats)
    return _CACHE[key]


def make_in_maps(x, wq, wk, wv, wo):
    """Build the 8 per-core input dicts from full inputs."""
    x = np.ascontiguousarray(np.asarray(x), dtype=np.float32)
    wqt = np.ascontiguousarray(np.asarray(wq).T, dtype=np.float32)
    wkt = np.ascontiguousarray(np.asarray(wk).T, dtype=np.float32)
    wvt = np.ascontiguousarray(np.asarray(wv).T, dtype=np.float32)
    wot = np.ascontiguousarray(np.asarray(wo).T, dtype=np.float32)
    in_maps = []
    for c in range(N_CORES):
        b, q0 = c // 4, (c % 4) * NQ
        xtb = np.ascontiguousarray(x[b].T)
        in_maps.append({
            "xt": xtb,
            "xtq": np.ascontiguousarray(xtb[:, q0:q0 + NQ]),
            "wqt": wqt, "wkt": wkt, "wvt": wvt, "wot": wot,
        })
    return in_maps


def assemble(results):
    """Concatenate per-core y outputs into the full [B, S, D] array."""
    out = np.empty((B, S, D), dtype=np.float32)
    for c in range(N_CORES):
        b, q0 = c // 4, (c % 4) * NQ
        out[b, q0:q0 + NQ, :] = results[c]["y"]
    return out


def _make_runner(nc):
    """Jit the 8-core shard_map execution once and reuse it across calls
    (mirrors bass2jax.run_bass_via_pjrt, which re-traces on every call)."""
    import jax
    import numpy as _np
    from jax.sharding import Mesh, PartitionSpec
    from jax.experimental.shard_map import shard_map

    import concourse.mybir as mybir
    from concourse import bass2jax

    bass2jax.install_neuronx_cc_hook()

    partition_name = nc.partition_id_tensor.name if nc.partition_id_tensor else None
    in_names, out_names, out_avals = [], [], []
    for alloc in nc.m.functions[0].allocations:
        if not isinstance(alloc, mybir.MemoryLocationSet):
            continue
        name = alloc.memorylocations[0].name
        if alloc.kind == "ExternalInput":
            if name != partition_name:
                in_names.append(name)
        elif alloc.kind == "ExternalOutput":
            out_names.append(name)
            out_avals.append(
                jax.core.ShapedArray(tuple(alloc.tensor_shape), mybir.dt.np(alloc.dtype))
            )
    n_params = len(in_names)
    all_in_names = list(in_names) + list(out_names)
    if partition_name is not None:
        all_in_names.append(partition_name)

    def _body(*args):
        operands = list(args)
        if partition_name is not None:
            operands.append(bass2jax.partition_id_tensor())
        return tuple(bass2jax._bass_exec_p.bind(
            *operands,
            out_avals=tuple(out_avals),
            in_names=tuple(all_in_names),
            out_names=tuple(out_names),
            lowering_input_output_aliases=(),
            sim_require_finite=True,
            sim_require_nnan=True,
            nc=nc,
        ))

    devices = jax.devices()[:N_CORES]
    mesh = Mesh(_np.asarray(devices), ("core",))
    specs = (PartitionSpec("core"),) * (n_params + len(out_names))
    fn = jax.jit(
        shard_map(_body, mesh=mesh, in_specs=specs,
                  out_specs=(PartitionSpec("core"),) * len(out_names),
                  check_rep=False),
        keep_unused=True,
    )

    def run(in_maps):
        concat_in = [
            _np.concatenate([_np.asarray(in_maps[c][nm]) for c in range(N_CORES)], axis=0)
            for nm in in_names
        ]
        concat_zero = [
            _np.zeros((N_CORES * a.shape[0], *a.shape[1:]), a.dtype) for a in out_avals
        ]
        outs = fn(*concat_in, *concat_zero)
        return [
            {nm: _np.asarray(outs[i]).reshape(N_CORES, *out_avals[i].shape)[c]
             for i, nm in enumerate(out_names)}
            for c in range(N_CORES)
        ]

    return run


def kernel(x, wq, wk, wv, wo):
    nc = _get_nc()
    in_maps = make_in_maps(x, wq, wk, wv, wo)
    try:
        if "runner" not in _CACHE:
            _CACHE["runner"] = _make_runner(nc)
        results = _CACHE["runner"](in_maps)
    except Exception:
        from concourse import bass_utils

        res = bass_utils.run_bass_kernel_spmd(
            nc, in_maps, core_ids=list(range(N_CORES))
        )
        results = res.results
    return assemble(results)
